# revision 27
# baseline (speedup 1.0000x reference)
"""Causal self-attention (B=4, T=2048, D=1024, H=16, hd=64) on 8 trn2 NeuronCores.

Sharding: data parallel over batch (4) x tensor parallel over heads (2 groups
of 8). Core c handles batch c//2 and heads (c%2)*8 .. (c%2)*8+8.
Wq/Wk/Wv are column-parallel by head group, Wo row-parallel; the pair of
cores sharing a batch produce partial outputs that are summed on the host.

On-device layout (per core) is fully "transposed": projections produce
Q^T, K^T [512, 2048] and V [2048, 512], scores are computed as
S^T = K Q^T (j=key on partitions, i=query on free dim), softmax uses
exp without max subtraction (scores are O(6) here), the denominator
comes for free from a ones-column appended to V, and attention output
O^T [hd, T] feeds the row-parallel out-projection directly as lhsT.

Head pairs share one [128, 1024] exp; their S^T matmuls row-pack onto
the PE concurrently (partition offsets 0/64). The per-chunk emission is
software-pipelined (S of tile jt+1 ahead of AV of tile jt in the PE
stream) so the PE never waits on the scalar engine's exp.
"""

import contextlib
import ctypes
import sys
import types

import numpy as np

B, T, D = 4, 2048, 1024
H_TOT, HD = 16, 64
SCALE = HD ** -0.5
P = 128
NH = 8            # heads per core
QD = NH * HD      # 512, projected dim per core
KT = D // P       # 8 contraction tiles for projections
MT = QD // P      # 4 qdim tiles
TT = T // P       # 16 token tiles
ACH = 512         # phase-A1 token chunk (Q/K); PSUM bank caps matmul N at 512
NACH = T // ACH   # 4
ICH = 512         # attention query chunk
NIC = T // ICH    # 4

_PROGRAM = None  # compiled program cache — build once per process


def _install_ntff_hook():
    """antenv.axon_hooks is missing in this image; recreate it so
    run_bass_kernel_spmd(trace=True) can profile. Harmless if unused."""
    if "antenv.axon_hooks" in sys.modules:
        return
    try:
        import antenv
    except ImportError:
        return
    mod = types.ModuleType("antenv.axon_hooks")
    _hook = [None]
    mod.set_axon_ntff_profile_hook = lambda h: _hook.__setitem__(0, h)
    mod.get_axon_ntff_profile_hook = lambda: _hook[0]
    antenv.axon_hooks = mod
    sys.modules["antenv.axon_hooks"] = mod
    try:
        lib = ctypes.CDLL("/opt/axon/libaxon_pjrt.so")
        if not hasattr(lib, "axon_start_nrt_profile"):
            return
        lib.axon_start_nrt_profile.argtypes = [
            ctypes.POINTER(ctypes.c_int64), ctypes.c_size_t]
        lib.axon_start_nrt_profile.restype = ctypes.c_int64
        lib.axon_stop_nrt_profile.argtypes = [ctypes.c_char_p]
        lib.axon_stop_nrt_profile.restype = ctypes.c_int64

        @contextlib.contextmanager
        def _hookfn(output_dir, device_ids):
            import jax
            jax.devices()
            if device_ids:
                ids = (ctypes.c_int64 * len(device_ids))(*device_ids)
                rc = lib.axon_start_nrt_profile(ids, len(device_ids))
            else:
                rc = lib.axon_start_nrt_profile(None, 0)
            if rc != 0:
                raise RuntimeError(f"axon_start_nrt_profile rc={rc}")
            try:
                yield
            finally:
                n = lib.axon_stop_nrt_profile(str(output_dir).encode())
                print(f"profile: {n} file(s) written to {output_dir}")

        mod.set_axon_ntff_profile_hook(_hookfn)
    except OSError:
        pass


def _build_program():
    from contextlib import ExitStack

    import concourse.tile as tile
    from concourse import bacc, mybir

    F32 = mybir.dt.float32
    BF16 = mybir.dt.bfloat16
    AF = mybir.ActivationFunctionType
    ALU = mybir.AluOpType

    nc = bacc.Bacc("TRN2", target_bir_lowering=False, debug=False,
                   num_devices=8)

    # all tensor inputs arrive pre-arranged in SBUF layout [128, k, n]
    # (host does the transpose) so every DMA is long contiguous runs
    xT_d = nc.dram_tensor("xT", [P, KT * T], BF16, kind="ExternalInput").ap()
    wq_d = nc.dram_tensor("wq", [P, KT * QD], BF16, kind="ExternalInput").ap()
    wk_d = nc.dram_tensor("wk", [P, KT * QD], BF16, kind="ExternalInput").ap()
    wv_d = nc.dram_tensor("wv", [P, KT * QD], BF16, kind="ExternalInput").ap()
    wo_d = nc.dram_tensor("wo", [P, MT * D], BF16, kind="ExternalInput").ap()
    bq_d = nc.dram_tensor("bq", [P, MT], F32, kind="ExternalInput").ap()
    bk_d = nc.dram_tensor("bk", [P, MT], F32, kind="ExternalInput").ap()
    bv_d = nc.dram_tensor("bv", [1, QD], F32, kind="ExternalInput").ap()
    msk_d = nc.dram_tensor("msk", [P, P], BF16, kind="ExternalInput").ap()
    out_d = nc.dram_tensor("out", [T, D], BF16, kind="ExternalOutput").ap()

    xT_k = xT_d.rearrange("p (k t) -> p k t", k=KT)      # [128, 8, 2048]
    wq_k = wq_d.rearrange("p (k m) -> p k m", k=KT)      # [128, 8, 512]
    wk_k = wk_d.rearrange("p (k m) -> p k m", k=KT)
    wv_k = wv_d.rearrange("p (k m) -> p k m", k=KT)
    wo_k = wo_d.rearrange("p (k e) -> p k e", k=MT)      # [128, 4, 1024]

    with tile.TileContext(nc) as tc, ExitStack() as ctx:
        persist = ctx.enter_context(tc.tile_pool(name="persist", bufs=1))

        qt = [persist.tile([P, T], BF16, name=f"qt{i}") for i in range(MT)]
        kt_ = [persist.tile([P, T], BF16, name=f"kt{i}") for i in range(MT)]
        v3 = [persist.tile([P, NH, HD + 1], BF16, name=f"v3_{i}")
              for i in range(TT)]
        at = [persist.tile([P, T], BF16, name=f"at{i}") for i in range(MT)]
        xt_all = persist.tile([P, KT, T], BF16, name="xt")

        wq_sb = persist.tile([P, KT, QD], BF16, name="wq")
        wk_sb = persist.tile([P, KT, QD], BF16, name="wk")
        bq_sb = persist.tile([P, MT], F32, name="bq")
        bk_sb = persist.tile([P, MT], F32, name="bk")
        bv_sb = persist.tile([1, QD], F32, name="bv")
        bvb = persist.tile([P, QD], F32, name="bvb")
        tri_sb = persist.tile([P, P], BF16, name="tri")
        wv_sb = persist.tile([P, KT, QD], BF16, name="wv")
        wo_sb = persist.tile([P, MT, D], BF16, name="wo")

        nc.sync.dma_start(wq_sb[:], wq_k)
        nc.sync.dma_start(xt_all[:, :, 0:ACH], xT_k[:, :, 0:ACH])
        nc.sync.dma_start(wk_sb[:], wk_d.rearrange("p (k m) -> p k m", k=KT))
        nc.sync.dma_start(bq_sb[:], bq_d)
        nc.sync.dma_start(bk_sb[:], bk_d)
        nc.sync.dma_start(xt_all[:, :, ACH:2 * ACH], xT_k[:, :, ACH:2 * ACH])
        nc.sync.dma_start(bv_sb[:], bv_d)
        nc.sync.dma_start(tri_sb[:], msk_d)
        nc.sync.dma_start(xt_all[:, :, 2 * ACH:3 * ACH],
                          xT_k[:, :, 2 * ACH:3 * ACH])
        nc.sync.dma_start(wv_sb[:], wv_k)
        nc.sync.dma_start(xt_all[:, :, 3 * ACH:4 * ACH],
                          xT_k[:, :, 3 * ACH:4 * ACH])
        nc.sync.dma_start(wo_sb[:], wo_k)
        # broadcast V bias across partitions once (the bias-add is fused
        # into the PSUM->SBUF V copy, replacing the ones-row bias matmul)
        nc.gpsimd.partition_broadcast(bvb[:], bv_sb[:])
        for tt in range(TT):
            nc.vector.memset(v3[tt][:, :, HD:HD + 1], 1.0)

        # ---- phase A1: Q^T, K^T projections for query/key quarter 0 only ---
        # (the remaining quarters are projected as PE fillers inside the
        # attention chunks, so the scalar engine's exp stream starts ~40us
        # earlier and the PE never drains at chunk boundaries)
        with tc.tile_pool(name="pjps1", bufs=1, space="PSUM") as pjp:
            for mt in range(MT):
                for w_sb, dst, b_sb in ((wq_sb, qt, bq_sb),
                                        (wk_sb, kt_, bk_sb)):
                    ps = pjp.tile([P, ACH], F32, name="pj", bufs=4)
                    for k in range(KT):
                        nc.tensor.matmul(
                            ps[:],
                            w_sb[:, k, mt * P:(mt + 1) * P],
                            xt_all[:, k, 0:ACH],
                            start=(k == 0), stop=(k == KT - 1))
                    nc.vector.tensor_scalar_add(dst[mt][:, 0:ACH],
                                                ps[:],
                                                b_sb[:, mt:mt + 1])

        # ---- phases A2/B/C interleaved per query chunk ---------------------
        with tc.tile_pool(name="attnsb", bufs=1) as ap_, \
             tc.tile_pool(name="obp", bufs=3) as obp, \
             tc.tile_pool(name="attnps", bufs=1, space="PSUM") as sp:

            def emit_qk_group(nch, mt, which):
                w_sb, dst, b_sb = ((wq_sb, qt, bq_sb) if which == 0
                                   else (wk_sb, kt_, bk_sb))
                ps = sp.tile([P, ACH], F32, name="misc", bufs=1)
                for k in range(KT):
                    nc.tensor.matmul(
                        ps[:], w_sb[:, k, mt * P:(mt + 1) * P],
                        xt_all[:, k, nch * ACH:(nch + 1) * ACH],
                        start=(k == 0), stop=(k == KT - 1))
                nc.vector.tensor_scalar_add(
                    dst[mt][:, nch * ACH:(nch + 1) * ACH], ps[:],
                    b_sb[:, mt:mt + 1])

            def emit_v_tile(tt):
                psv = sp.tile([P, QD], F32, name="misc", bufs=1)
                for k in range(KT):
                    nc.tensor.matmul(
                        psv[:], xt_all[:, k, tt * P:(tt + 1) * P],
                        wv_sb[:, k, :], start=(k == 0), stop=(k == KT - 1))
                nc.vector.tensor_tensor(
                    v3[tt][:, :, 0:HD],
                    psv[:].rearrange("p (h d) -> p h d", d=HD),
                    bvb[:].rearrange("p (h d) -> p h d", d=HD),
                    op=ALU.add)

            def emit_attn_chunk(ic, fillers=()):
                """Attention for query chunk ic, as 4 head-pairs, with the
                PE stream software-pipelined: S of j-tile jt+1 is emitted
                before AV of j-tile jt so exp latency is hidden. One filler
                (a V-tile projection or an out-projection group for another
                chunk) is emitted per head-pair boundary to keep the PE fed
                while the scalar engine works through the exps."""
                isl = slice(ic * ICH, (ic + 1) * ICH)
                njt = 4 * ic + 4
                fillers = list(fillers)
                pending = []
                for hp in range(MT):
                    opsA = sp.tile([HD + 1, ICH], F32, name="opsum", bufs=3)
                    opsB = sp.tile([HD + 1, ICH], F32, name="opsum", bufs=3)
                    s2s, e2s = {}, {}

                    def emit_s(jt):
                        # diagonal tiles restrict the matmul stream to the
                        # causally valid query range (width w); head B's
                        # scores sit bank-aligned at offset ICH
                        kdiag = jt - 4 * ic
                        c0 = max(kdiag, 0) * P
                        w = ICH - c0
                        s2 = sp.tile([P, 2 * ICH], F32, name="spsum", bufs=2)
                        jsl = slice(jt * P, (jt + 1) * P)
                        qsl = slice(ic * ICH + c0, (ic + 1) * ICH)
                        nc.tensor.matmul(s2[:, 0:w], kt_[hp][0:HD, jsl],
                                         qt[hp][0:HD, qsl],
                                         start=True, stop=True)
                        nc.tensor.matmul(s2[:, ICH:ICH + w],
                                         kt_[hp][HD:P, jsl],
                                         qt[hp][HD:P, qsl],
                                         start=True, stop=True)
                        s2s[jt] = s2

                    def emit_exp(jt):
                        kdiag = jt - 4 * ic
                        w = ICH - max(kdiag, 0) * P
                        e2 = ap_.tile([P, 2 * ICH], BF16, name="e", bufs=3)
                        s3 = s2s.pop(jt)[:].rearrange("p (o i) -> p o i",
                                                      o=2)
                        e3 = e2[:].rearrange("p (o i) -> p o i", o=2)
                        nc.scalar.activation(e3[:, :, 0:w], s3[:, :, 0:w],
                                             AF.Exp)
                        if kdiag >= 0:
                            # zero the diagonal block's upper triangle
                            # (local cols 0:P of the restricted range)
                            for half in range(2):
                                o = half * ICH
                                nc.vector.tensor_tensor(
                                    e2[:, o:o + P], e2[:, o:o + P],
                                    tri_sb[:], op=ALU.mult)
                        e2s[jt] = e2

                    def emit_av(jt):
                        # columns left of the diagonal block are causally
                        # invalid — S/exp/AV all restricted to width w
                        kdiag = jt - 4 * ic
                        c0 = max(kdiag, 0) * P
                        w = ICH - c0
                        e2 = e2s.pop(jt)
                        nc.tensor.matmul(opsA[:, c0:], v3[jt][:, 2 * hp, :],
                                         e2[:, 0:w],
                                         start=(jt == 0),
                                         stop=(jt == njt - 1))
                        nc.tensor.matmul(opsB[:, c0:],
                                         v3[jt][:, 2 * hp + 1, :],
                                         e2[:, ICH:ICH + w],
                                         start=(jt == 0),
                                         stop=(jt == njt - 1))

                    emit_s(0)
                    for jt in range(1, njt):
                        if jt == 2 and pending:
                            pending.pop()()
                        if fillers:
                            # PE filler BEFORE the next S matmul: the filler
                            # stream absorbs the wait on the spsum slot's
                            # previous reader (the exp two tiles back), so
                            # the S matmul's weight load is never exposed
                            fillers.pop(0)()
                        emit_s(jt)
                        emit_exp(jt - 1)
                        emit_av(jt - 1)
                    emit_exp(njt - 1)
                    if fillers:
                        # hide the final exp's latency behind independent work
                        fillers.pop(0)()
                    emit_av(njt - 1)

                    if ic == 0 and pending:
                        pending.pop()()

                    def normalize(hp=hp, opsA=opsA, opsB=opsB):
                        # normalize straight out of PSUM: in0 is PSUM so the
                        # SBUF base-partition pairing rule doesn't apply
                        for half, ops in ((0, opsA), (1, opsB)):
                            po = half * HD
                            dn = ap_.tile([1, ICH], F32, name="dn", bufs=4)
                            nc.vector.tensor_copy(dn[:], ops[HD:HD + 1, :])
                            recip = ap_.tile([1, ICH], F32, name="recip",
                                             bufs=4)
                            nc.vector.reciprocal_approx_fast(recip[:], dn[:])
                            rb = ap_.tile([HD, ICH], F32, name="rb", bufs=4)
                            nc.gpsimd.partition_broadcast(rb[:], recip[:])
                            nc.vector.tensor_tensor(
                                at[hp][po:po + HD, isl], ops[0:HD, :], rb[:],
                                op=ALU.mult)

                    pending.append(normalize)

                while pending:
                    pending.pop()()
                for f in fillers:
                    f()

            def emit_out_group(mt, nch2, alt=False):
                if alt:
                    # tail-only: borrow an (idle by then) spsum-tag slot so
                    # consecutive groups double-buffer instead of serializing
                    pso = sp.tile([P, 2 * ICH], F32, name="spsum",
                                  bufs=2)[:, 0:512]
                else:
                    pso = sp.tile([P, 512], F32, name="misc", bufs=1)
                for k in range(MT):
                    nc.tensor.matmul(
                        pso[:], at[k][:, mt * P:(mt + 1) * P],
                        wo_sb[:, k, nch2 * 512:(nch2 + 1) * 512],
                        start=(k == 0), stop=(k == MT - 1))
                ob = obp.tile([P, 512], BF16, name="ob")
                nc.vector.tensor_copy(ob[:], pso[:])
                nc.sync.dma_start(
                    out_d[mt * P:(mt + 1) * P,
                          nch2 * 512:(nch2 + 1) * 512], ob[:])

            # out-projection groups of chunk ic are spread over chunks
            # ic+1 and ic+2 (half each) so the ACT-bound late chunks keep
            # a PE filler inventory
            out_sched = {1: [(mt, n) for mt in range(0, 2) for n in range(2)],
                         2: [(mt, n) for mt in range(2, 6) for n in range(2)],
                         3: [(mt, n) for mt in range(6, 12) for n in range(2)]}
            for tt in range(4):
                emit_v_tile(tt)
            for ic in range(NIC):
                fillers = []
                if ic + 1 < NIC:
                    # project Q/K quarter ic+1 (needed by the next chunk)
                    fillers += [
                        (lambda mt=mt, w=w: emit_qk_group(ic + 1, mt, w))
                        for mt in range(MT) for w in range(2)]
                    fillers += [
                        (lambda tt=tt: emit_v_tile(tt))
                        for tt in range(4 * ic + 4, 4 * ic + 8)]
                fillers += [
                    (lambda mt=mt, n=n: emit_out_group(mt, n))
                    for mt, n in out_sched.get(ic, [])]
                emit_attn_chunk(ic, fillers)
            for i, (mt, n) in enumerate(
                    (mt, n) for mt in range(12, 16) for n in range(2)):
                emit_out_group(mt, n, alt=(i % 2 == 1))

    nc.compile()
    return nc


def _get_program():
    global _PROGRAM
    if _PROGRAM is None:
        _install_ntff_hook()
        _PROGRAM = _build_program()
    return _PROGRAM


def _make_masks():
    """Multiplicative upper-triangle zero mask [128, 128] for the diagonal
    128x128 block of each S^T tile: entry (j, i) = 1 if j <= i else 0."""
    j = np.arange(P)[:, None]
    i = np.arange(P)[None, :]
    return (j <= i).astype(np.float32)


def make_in_maps(x, Wq, bq, Wk, bk, Wv, bv, Wo, bo):
    import ml_dtypes
    bf16 = ml_dtypes.bfloat16

    def sbl(a, k):
        """[k*128, n] -> SBUF layout [128, k*n] (partition-major runs)."""
        n = a.shape[1]
        return np.ascontiguousarray(
            a.reshape(k, P, n).transpose(1, 0, 2).reshape(P, k * n)
        ).astype(bf16)

    masks = _make_masks()
    in_maps = []
    for c in range(8):
        b, hg = c // 2, c % 2
        sl = slice(hg * QD, (hg + 1) * QD)
        in_maps.append({
            "xT": sbl(np.ascontiguousarray(x[b].T), KT),
            "wq": sbl(Wq[:, sl] * SCALE, KT),
            "wk": sbl(Wk[:, sl], KT),
            "wv": sbl(Wv[:, sl], KT),
            "wo": sbl(Wo[sl, :], MT),
            "bq": np.ascontiguousarray((bq[sl] * SCALE).reshape(MT, P).T),
            "bk": np.ascontiguousarray(bk[sl].reshape(MT, P).T),
            "bv": np.ascontiguousarray(
                bv[sl].reshape(1, QD)).astype(np.float32),
            "msk": masks.astype(bf16),
        })
    return in_maps


def run(inputs, trace=False):
    from concourse.bass_utils import run_bass_kernel_spmd

    nc = _get_program()
    in_maps = make_in_maps(**inputs)
    res = run_bass_kernel_spmd(nc, in_maps, list(range(8)), trace=trace)
    bo = inputs["bo"]
    out = np.empty((B, T, D), dtype=np.float32)
    for b in range(B):
        out[b] = (res.results[2 * b]["out"].astype(np.float32)
                  + res.results[2 * b + 1]["out"].astype(np.float32) + bo)
    return out, res


def kernel(**inputs):
    inputs = {k: np.asarray(v) for k, v in inputs.items()}
    out, _ = run(inputs)
    return out



# revision 29
# speedup vs baseline: 1.0024x; 1.0024x over previous
"""Causal self-attention (B=4, T=2048, D=1024, H=16, hd=64) on 8 trn2 NeuronCores.

Sharding: data parallel over batch (4) x tensor parallel over heads (2 groups
of 8). Core c handles batch c//2 and heads (c%2)*8 .. (c%2)*8+8.
Wq/Wk/Wv are column-parallel by head group, Wo row-parallel; the pair of
cores sharing a batch produce partial outputs that are summed on the host.

On-device layout (per core) is fully "transposed": projections produce
Q^T, K^T [512, 2048] and V [2048, 512], scores are computed as
S^T = K Q^T (j=key on partitions, i=query on free dim), softmax uses
exp without max subtraction (scores are O(6) here), the denominator
comes for free from a ones-column appended to V, and attention output
O^T [hd, T] feeds the row-parallel out-projection directly as lhsT.

Head pairs share one [128, 1024] exp; their S^T matmuls row-pack onto
the PE concurrently (partition offsets 0/64). The per-chunk emission is
software-pipelined (S of tile jt+1 ahead of AV of tile jt in the PE
stream) so the PE never waits on the scalar engine's exp.
"""

import contextlib
import ctypes
import sys
import types

import numpy as np

B, T, D = 4, 2048, 1024
H_TOT, HD = 16, 64
SCALE = HD ** -0.5
P = 128
NH = 8            # heads per core
QD = NH * HD      # 512, projected dim per core
KT = D // P       # 8 contraction tiles for projections
MT = QD // P      # 4 qdim tiles
TT = T // P       # 16 token tiles
ACH = 512         # phase-A1 token chunk (Q/K); PSUM bank caps matmul N at 512
NACH = T // ACH   # 4
ICH = 512         # attention query chunk
NIC = T // ICH    # 4

_PROGRAM = None  # compiled program cache — build once per process


def _install_ntff_hook():
    """antenv.axon_hooks is missing in this image; recreate it so
    run_bass_kernel_spmd(trace=True) can profile. Harmless if unused."""
    if "antenv.axon_hooks" in sys.modules:
        return
    try:
        import antenv
    except ImportError:
        return
    mod = types.ModuleType("antenv.axon_hooks")
    _hook = [None]
    mod.set_axon_ntff_profile_hook = lambda h: _hook.__setitem__(0, h)
    mod.get_axon_ntff_profile_hook = lambda: _hook[0]
    antenv.axon_hooks = mod
    sys.modules["antenv.axon_hooks"] = mod
    try:
        lib = ctypes.CDLL("/opt/axon/libaxon_pjrt.so")
        if not hasattr(lib, "axon_start_nrt_profile"):
            return
        lib.axon_start_nrt_profile.argtypes = [
            ctypes.POINTER(ctypes.c_int64), ctypes.c_size_t]
        lib.axon_start_nrt_profile.restype = ctypes.c_int64
        lib.axon_stop_nrt_profile.argtypes = [ctypes.c_char_p]
        lib.axon_stop_nrt_profile.restype = ctypes.c_int64

        @contextlib.contextmanager
        def _hookfn(output_dir, device_ids):
            import jax
            jax.devices()
            if device_ids:
                ids = (ctypes.c_int64 * len(device_ids))(*device_ids)
                rc = lib.axon_start_nrt_profile(ids, len(device_ids))
            else:
                rc = lib.axon_start_nrt_profile(None, 0)
            if rc != 0:
                raise RuntimeError(f"axon_start_nrt_profile rc={rc}")
            try:
                yield
            finally:
                n = lib.axon_stop_nrt_profile(str(output_dir).encode())
                print(f"profile: {n} file(s) written to {output_dir}")

        mod.set_axon_ntff_profile_hook(_hookfn)
    except OSError:
        pass


def _build_program():
    from contextlib import ExitStack

    import concourse.tile as tile
    from concourse import bacc, mybir

    F32 = mybir.dt.float32
    BF16 = mybir.dt.bfloat16
    AF = mybir.ActivationFunctionType
    ALU = mybir.AluOpType

    nc = bacc.Bacc("TRN2", target_bir_lowering=False, debug=False,
                   num_devices=8)

    # all tensor inputs arrive pre-arranged in SBUF layout [128, k, n]
    # (host does the transpose) so every DMA is long contiguous runs
    xT_d = nc.dram_tensor("xT", [P, KT * T], BF16, kind="ExternalInput").ap()
    wq_d = nc.dram_tensor("wq", [P, KT * QD], BF16, kind="ExternalInput").ap()
    wk_d = nc.dram_tensor("wk", [P, KT * QD], BF16, kind="ExternalInput").ap()
    wv_d = nc.dram_tensor("wv", [P, KT * QD], BF16, kind="ExternalInput").ap()
    wo_d = nc.dram_tensor("wo", [P, MT * D], BF16, kind="ExternalInput").ap()
    bq_d = nc.dram_tensor("bq", [P, MT], F32, kind="ExternalInput").ap()
    bk_d = nc.dram_tensor("bk", [P, MT], F32, kind="ExternalInput").ap()
    bv_d = nc.dram_tensor("bv", [1, QD], F32, kind="ExternalInput").ap()
    msk_d = nc.dram_tensor("msk", [P, P], BF16, kind="ExternalInput").ap()
    out_d = nc.dram_tensor("out", [T, D], BF16, kind="ExternalOutput").ap()

    xT_k = xT_d.rearrange("p (k t) -> p k t", k=KT)      # [128, 8, 2048]
    wq_k = wq_d.rearrange("p (k m) -> p k m", k=KT)      # [128, 8, 512]
    wk_k = wk_d.rearrange("p (k m) -> p k m", k=KT)
    wv_k = wv_d.rearrange("p (k m) -> p k m", k=KT)
    wo_k = wo_d.rearrange("p (k e) -> p k e", k=MT)      # [128, 4, 1024]

    with tile.TileContext(nc) as tc, ExitStack() as ctx:
        persist = ctx.enter_context(tc.tile_pool(name="persist", bufs=1))

        qt = [persist.tile([P, T], BF16, name=f"qt{i}") for i in range(MT)]
        kt_ = [persist.tile([P, T], BF16, name=f"kt{i}") for i in range(MT)]
        v3 = [persist.tile([P, NH, HD + 1], BF16, name=f"v3_{i}")
              for i in range(TT)]
        at = [persist.tile([P, T], BF16, name=f"at{i}") for i in range(MT)]
        xt_all = persist.tile([P, KT, T], BF16, name="xt")

        wq_sb = persist.tile([P, KT, QD], BF16, name="wq")
        wk_sb = persist.tile([P, KT, QD], BF16, name="wk")
        bq_sb = persist.tile([P, MT], F32, name="bq")
        bk_sb = persist.tile([P, MT], F32, name="bk")
        bv_sb = persist.tile([1, QD], F32, name="bv")
        bvb = persist.tile([P, QD], F32, name="bvb")
        tri_sb = persist.tile([P, P], BF16, name="tri")
        wv_sb = persist.tile([P, KT, QD], BF16, name="wv")
        wo_sb = persist.tile([P, MT, D], BF16, name="wo")

        nc.sync.dma_start(wq_sb[:], wq_k)
        nc.sync.dma_start(xt_all[:, :, 0:ACH], xT_k[:, :, 0:ACH])
        nc.sync.dma_start(wk_sb[:], wk_d.rearrange("p (k m) -> p k m", k=KT))
        nc.sync.dma_start(bq_sb[:], bq_d)
        nc.sync.dma_start(bk_sb[:], bk_d)
        nc.sync.dma_start(xt_all[:, :, ACH:2 * ACH], xT_k[:, :, ACH:2 * ACH])
        nc.sync.dma_start(bv_sb[:], bv_d)
        nc.sync.dma_start(tri_sb[:], msk_d)
        nc.sync.dma_start(xt_all[:, :, 2 * ACH:3 * ACH],
                          xT_k[:, :, 2 * ACH:3 * ACH])
        nc.sync.dma_start(wv_sb[:], wv_k)
        nc.sync.dma_start(xt_all[:, :, 3 * ACH:4 * ACH],
                          xT_k[:, :, 3 * ACH:4 * ACH])
        nc.sync.dma_start(wo_sb[:], wo_k)
        # broadcast V bias across partitions once (the bias-add is fused
        # into the PSUM->SBUF V copy, replacing the ones-row bias matmul)
        nc.gpsimd.partition_broadcast(bvb[:], bv_sb[:])
        for tt in range(TT):
            nc.vector.memset(v3[tt][:, :, HD:HD + 1], 1.0)

        # ---- phase A1: Q^T, K^T projections for query/key quarter 0 only ---
        # (the remaining quarters are projected as PE fillers inside the
        # attention chunks, so the scalar engine's exp stream starts ~40us
        # earlier and the PE never drains at chunk boundaries)
        with tc.tile_pool(name="pjps1", bufs=1, space="PSUM") as pjp:
            for mt in range(MT):
                for w_sb, dst, b_sb in ((wq_sb, qt, bq_sb),
                                        (wk_sb, kt_, bk_sb)):
                    ps = pjp.tile([P, ACH], F32, name="pj", bufs=4)
                    for k in range(KT):
                        nc.tensor.matmul(
                            ps[:],
                            w_sb[:, k, mt * P:(mt + 1) * P],
                            xt_all[:, k, 0:ACH],
                            start=(k == 0), stop=(k == KT - 1))
                    nc.vector.tensor_scalar_add(dst[mt][:, 0:ACH],
                                                ps[:],
                                                b_sb[:, mt:mt + 1])

        # ---- phases A2/B/C interleaved per query chunk ---------------------
        with tc.tile_pool(name="attnsb", bufs=1) as ap_, \
             tc.tile_pool(name="obp", bufs=3) as obp, \
             tc.tile_pool(name="attnps", bufs=1, space="PSUM") as sp:

            def emit_qk_group(nch, mt, which):
                w_sb, dst, b_sb = ((wq_sb, qt, bq_sb) if which == 0
                                   else (wk_sb, kt_, bk_sb))
                ps = sp.tile([P, ACH], F32, name="misc", bufs=1)
                for k in range(KT):
                    nc.tensor.matmul(
                        ps[:], w_sb[:, k, mt * P:(mt + 1) * P],
                        xt_all[:, k, nch * ACH:(nch + 1) * ACH],
                        start=(k == 0), stop=(k == KT - 1))
                nc.vector.tensor_scalar_add(
                    dst[mt][:, nch * ACH:(nch + 1) * ACH], ps[:],
                    b_sb[:, mt:mt + 1])

            def emit_v_tile(tt):
                psv = sp.tile([P, QD], F32, name="misc", bufs=1)
                for k in range(KT):
                    nc.tensor.matmul(
                        psv[:], xt_all[:, k, tt * P:(tt + 1) * P],
                        wv_sb[:, k, :], start=(k == 0), stop=(k == KT - 1))
                nc.vector.tensor_tensor(
                    v3[tt][:, :, 0:HD],
                    psv[:].rearrange("p (h d) -> p h d", d=HD),
                    bvb[:].rearrange("p (h d) -> p h d", d=HD),
                    op=ALU.add)

            pending = []

            def emit_attn_unit(ic, hp, fillers=()):
                """Attention for one (query chunk, head pair) unit: the
                j-tile pipeline (S -> exp -> AV) plus the deferred
                normalize. fillers keep the PE fed while the scalar
                engine works through the exps."""
                isl = slice(ic * ICH, (ic + 1) * ICH)
                njt = 4 * ic + 4
                fillers = list(fillers)
                opsA = sp.tile([HD + 1, ICH], F32, name="opsum", bufs=3)
                opsB = sp.tile([HD + 1, ICH], F32, name="opsum", bufs=3)
                s2s, e2s = {}, {}

                def emit_s(jt):
                    # diagonal tiles restrict the matmul stream to the
                    # causally valid query range (width w); head B's
                    # scores sit bank-aligned at offset ICH
                    kdiag = jt - 4 * ic
                    c0 = max(kdiag, 0) * P
                    w = ICH - c0
                    s2 = sp.tile([P, 2 * ICH], F32, name="spsum", bufs=2)
                    jsl = slice(jt * P, (jt + 1) * P)
                    qsl = slice(ic * ICH + c0, (ic + 1) * ICH)
                    nc.tensor.matmul(s2[:, 0:w], kt_[hp][0:HD, jsl],
                                     qt[hp][0:HD, qsl],
                                     start=True, stop=True)
                    nc.tensor.matmul(s2[:, ICH:ICH + w],
                                     kt_[hp][HD:P, jsl],
                                     qt[hp][HD:P, qsl],
                                     start=True, stop=True)
                    s2s[jt] = s2

                def emit_exp(jt):
                    kdiag = jt - 4 * ic
                    w = ICH - max(kdiag, 0) * P
                    e2 = ap_.tile([P, 2 * ICH], BF16, name="e", bufs=3)
                    s3 = s2s.pop(jt)[:].rearrange("p (o i) -> p o i", o=2)
                    e3 = e2[:].rearrange("p (o i) -> p o i", o=2)
                    nc.scalar.activation(e3[:, :, 0:w], s3[:, :, 0:w],
                                         AF.Exp)
                    if kdiag >= 0:
                        # zero the diagonal block's upper triangle
                        # (local cols 0:P of the restricted range)
                        for half in range(2):
                            o = half * ICH
                            nc.vector.tensor_tensor(
                                e2[:, o:o + P], e2[:, o:o + P],
                                tri_sb[:], op=ALU.mult)
                    e2s[jt] = e2

                def emit_av(jt):
                    # columns left of the diagonal block are causally
                    # invalid — S/exp/AV all restricted to width w
                    kdiag = jt - 4 * ic
                    c0 = max(kdiag, 0) * P
                    w = ICH - c0
                    e2 = e2s.pop(jt)
                    nc.tensor.matmul(opsA[:, c0:], v3[jt][:, 2 * hp, :],
                                     e2[:, 0:w],
                                     start=(jt == 0),
                                     stop=(jt == njt - 1))
                    nc.tensor.matmul(opsB[:, c0:],
                                     v3[jt][:, 2 * hp + 1, :],
                                     e2[:, ICH:ICH + w],
                                     start=(jt == 0),
                                     stop=(jt == njt - 1))

                emit_s(0)
                for jt in range(1, njt):
                    if jt == 2 and pending:
                        pending.pop()()
                    if fillers:
                        # PE filler BEFORE the next S matmul: the filler
                        # stream absorbs the wait on the spsum slot's
                        # previous reader (the exp two tiles back), so
                        # the S matmul's weight load stays hidden
                        fillers.pop(0)()
                    emit_s(jt)
                    emit_exp(jt - 1)
                    emit_av(jt - 1)
                emit_exp(njt - 1)
                if fillers:
                    # hide the final exp's latency behind independent work
                    fillers.pop(0)()
                emit_av(njt - 1)

                def normalize(hp=hp, isl=isl, opsA=opsA, opsB=opsB):
                    # normalize straight out of PSUM: in0 is PSUM so the
                    # SBUF base-partition pairing rule doesn't apply
                    for half, ops in ((0, opsA), (1, opsB)):
                        po = half * HD
                        dn = ap_.tile([1, ICH], F32, name="dn", bufs=4)
                        nc.vector.tensor_copy(dn[:], ops[HD:HD + 1, :])
                        recip = ap_.tile([1, ICH], F32, name="recip",
                                         bufs=4)
                        nc.vector.reciprocal_approx_fast(recip[:], dn[:])
                        rb = ap_.tile([HD, ICH], F32, name="rb", bufs=4)
                        nc.gpsimd.partition_broadcast(rb[:], recip[:])
                        nc.vector.tensor_tensor(
                            at[hp][po:po + HD, isl], ops[0:HD, :], rb[:],
                            op=ALU.mult)

                for f in fillers:
                    f()
                pending.append(normalize)

            def emit_out_group(mt, nch2, alt=False):
                if alt:
                    # tail-only: borrow an (idle by then) spsum-tag slot so
                    # consecutive groups double-buffer instead of serializing
                    pso = sp.tile([P, 2 * ICH], F32, name="spsum",
                                  bufs=2)[:, 0:512]
                else:
                    pso = sp.tile([P, 512], F32, name="misc", bufs=1)
                for k in range(MT):
                    nc.tensor.matmul(
                        pso[:], at[k][:, mt * P:(mt + 1) * P],
                        wo_sb[:, k, nch2 * 512:(nch2 + 1) * 512],
                        start=(k == 0), stop=(k == MT - 1))
                ob = obp.tile([P, 512], BF16, name="ob")
                nc.vector.tensor_copy(ob[:], pso[:])
                nc.sync.dma_start(
                    out_d[mt * P:(mt + 1) * P,
                          nch2 * 512:(nch2 + 1) * 512], ob[:])

            # unit schedule: chunks 0 and 1 in order, then chunks 2 and 3
            # interleaved by head pair so the scalar-engine (exp) load is
            # flat instead of saturating at the end. Fillers are assigned
            # per unit, respecting availability: Q/K quarter q and V
            # tiles of quarter q are projected before the first unit that
            # reads them; out-projection groups of chunk c only after
            # chunk c's last normalize.
            QK = [(lambda mt=mt, q=q, w=w: emit_qk_group(q, mt, w))
                  for q in (1, 2, 3) for mt in range(MT) for w in range(2)]
            VT = [(lambda tt=tt: emit_v_tile(tt)) for tt in range(4, TT)]
            OG = [(lambda mt=mt, n=n: emit_out_group(mt, n))
                  for mt in range(12) for n in range(2)]
            qk1, qk2, qk3 = QK[0:8], QK[8:16], QK[16:24]
            v1, v2, v3f = VT[0:4], VT[4:8], VT[8:12]
            o0, o1, o2 = OG[0:8], OG[8:16], OG[16:24]
            schedule = [
                ((0, 0), qk1[0:2]), ((0, 1), qk1[2:4]),
                ((0, 2), qk1[4:6] + v1[0:1]), ((0, 3), qk1[6:8] + v1[1:2]),
                ((1, 0), v1[2:4] + qk2[0:1]), ((1, 1), qk2[1:4]),
                ((1, 2), qk2[4:7]), ((1, 3), qk2[7:8] + v2[0:3]),
                ((2, 0), v2[3:4] + qk3[0:4]), ((2, 1), qk3[4:8] + v3f[0:1]),
                ((3, 0), v3f[1:4] + o0[0:2]), ((2, 2), o0[2:6]),
                ((3, 1), o0[6:8] + o1[0:2]), ((2, 3), o1[2:6]),
                ((3, 2), o1[6:8]), ((3, 3), o2),
            ]
            for tt in range(4):
                emit_v_tile(tt)
            for (ic, hp), fillers in schedule:
                emit_attn_unit(ic, hp, fillers)
            while pending:
                pending.pop()()
            for i, (mt, n) in enumerate(
                    (mt, n) for mt in range(12, 16) for n in range(2)):
                emit_out_group(mt, n, alt=(i % 2 == 1))

    nc.compile()
    return nc


def _get_program():
    global _PROGRAM
    if _PROGRAM is None:
        _install_ntff_hook()
        _PROGRAM = _build_program()
    return _PROGRAM


def _make_masks():
    """Multiplicative upper-triangle zero mask [128, 128] for the diagonal
    128x128 block of each S^T tile: entry (j, i) = 1 if j <= i else 0."""
    j = np.arange(P)[:, None]
    i = np.arange(P)[None, :]
    return (j <= i).astype(np.float32)


def make_in_maps(x, Wq, bq, Wk, bk, Wv, bv, Wo, bo):
    import ml_dtypes
    bf16 = ml_dtypes.bfloat16

    def sbl(a, k):
        """[k*128, n] -> SBUF layout [128, k*n] (partition-major runs)."""
        n = a.shape[1]
        return np.ascontiguousarray(
            a.reshape(k, P, n).transpose(1, 0, 2).reshape(P, k * n)
        ).astype(bf16)

    masks = _make_masks()
    in_maps = []
    for c in range(8):
        b, hg = c // 2, c % 2
        sl = slice(hg * QD, (hg + 1) * QD)
        in_maps.append({
            "xT": sbl(np.ascontiguousarray(x[b].T), KT),
            "wq": sbl(Wq[:, sl] * SCALE, KT),
            "wk": sbl(Wk[:, sl], KT),
            "wv": sbl(Wv[:, sl], KT),
            "wo": sbl(Wo[sl, :], MT),
            "bq": np.ascontiguousarray((bq[sl] * SCALE).reshape(MT, P).T),
            "bk": np.ascontiguousarray(bk[sl].reshape(MT, P).T),
            "bv": np.ascontiguousarray(
                bv[sl].reshape(1, QD)).astype(np.float32),
            "msk": masks.astype(bf16),
        })
    return in_maps


def run(inputs, trace=False):
    from concourse.bass_utils import run_bass_kernel_spmd

    nc = _get_program()
    in_maps = make_in_maps(**inputs)
    res = run_bass_kernel_spmd(nc, in_maps, list(range(8)), trace=trace)
    bo = inputs["bo"]
    out = np.empty((B, T, D), dtype=np.float32)
    for b in range(B):
        out[b] = (res.results[2 * b]["out"].astype(np.float32)
                  + res.results[2 * b + 1]["out"].astype(np.float32) + bo)
    return out, res


def kernel(**inputs):
    inputs = {k: np.asarray(v) for k, v in inputs.items()}
    out, _ = run(inputs)
    return out

